# revision 1
# baseline (speedup 1.0000x reference)
"""Causal self-attention (B=4, T=4096, D=H=1024, fp32) on 8 Trainium2 cores.

Sharding: 2 cores per batch element. Within a batch, the 32 query tiles of
128 rows are interleaved between the 2 cores (core `pair` p takes global
q-tiles p, p+2, p+4, ...), which balances the causal-attention work exactly.
Each core computes the full K/V projection for its batch (replicated between
the 2 cores of a batch), then flash-style attention over its 16 q-tiles.

Numerics: x and the weights are cast to bf16 on the host; all matmuls run
bf16 with fp32 PSUM accumulation. Softmax skips max-subtraction (scores are
~N(0,1) after the 1/32 scale so exp stays in a safe fp32 range); exp runs on
ScalarE in fp32, probabilities are stored bf16, and the final normalization
is fp32. Measured error vs the fp32 reference: ~0.4% scale-relative absmax.
"""

import numpy as np

B, T, D, H = 4, 4096, 1024, 1024
P = 128
NCORES = 8


DEFAULT_CFG = dict(
    phases="AB",
    xpose="pe",           # "pe": PE transposes; "dma": xbar-transpose loads of x^T
    wq_top=False,         # preload Wq before phase A
    pa_xb_bufs=8, pa_xt_bufs=1,
    pa_pst_bufs=2, pa_psk_bufs=2, pa_psv_bufs=2,
    pb_xb_bufs=8, pb_xt_bufs=1, pb_qt_bufs=1,
    pb_p_bufs=3, pb_pt_bufs=2, pb_ob_bufs=2,
    pb_pp_bufs=3, pb_ps_bufs=3, pb_po_bufs=1,
    s_ahead=2,
)


def _emit(ctx, tc, xq, xkv, wq, wk, wv, maskt, ident, outp, T_kv, n_qt, cfg):
    import concourse.mybir as mybir

    nc = tc.nc
    f32 = mybir.dt.float32
    bf16 = mybir.dt.bfloat16
    Copy = mybir.ActivationFunctionType.Copy
    Exp = mybir.ActivationFunctionType.Exp
    AX = mybir.AxisListType.X
    SCALE = 1.0 / 32.0  # 1/sqrt(H)

    NKB = T_kv // 128     # kv 128-blocks
    NKC_A = T_kv // 512   # phase-A 512-row projection chunks
    NSC = n_qt // 4       # 512-row query superchunks
    if "A" not in cfg["phases"]:
        NKC_A = 0
    if "B" not in cfg["phases"]:
        NSC = 0

    const = ctx.enter_context(tc.tile_pool(name="const", bufs=1))
    persist = ctx.enter_context(tc.tile_pool(name="persist", bufs=1))

    id_sb = const.tile([P, P], bf16, tag="ident")
    nc.sync.dma_start(out=id_sb, in_=ident)
    mask_sb = const.tile([P, 256], bf16, tag="mask")
    nc.sync.dma_start(out=mask_sb, in_=maskt)

    # K^T laid out [h%128, h//128, t]; V laid out [t%128, t//128, h]
    KT = persist.tile([P, 8, T_kv], bf16, tag="KT")
    V = persist.tile([P, NKB, 1024], bf16, tag="V")

    def load_weight(wdram, wsb):
        # DRAM [1024,1024] bf16 -> SBUF [128, 8, 1024] (d = dc*128 + p)
        for dc in range(8):
            nc.sync.dma_start(out=wsb[:, dc, :], in_=wdram[dc * P:(dc + 1) * P, :])

    dma_xpose = cfg["xpose"] == "dma"
    wq_top = cfg["wq_top"]
    if wq_top:
        wq_sb_top = persist.tile([P, 8, 1024], bf16, tag="wq")
        load_weight(wq, wq_sb_top)

    def load_xt(xt, xsrc, r0, ps_pool, xb_pool, xb_tag):
        """Fill xt[:, dc, :] = x[r0:r0+512, dc*128:(dc+1)*128]^T for all dc."""
        if dma_xpose:
            for dc in range(8):
                nc.sync.dma_start_transpose(
                    out=xt[:, dc, :],
                    in_=xsrc[r0:r0 + 512, dc * P:(dc + 1) * P])
            return
        xbs = []
        for i in range(4):
            xb = xb_pool.tile([P, 1024], bf16, tag=xb_tag)
            nc.sync.dma_start(out=xb, in_=xsrc[r0 + i * P: r0 + (i + 1) * P, :])
            xbs.append(xb)
        for hf in range(2):
            for dc in range(8):
                tp = ps_pool.tile([P, 256], bf16, tag="pp")
                for i in range(2):
                    nc.tensor.transpose(
                        tp[:, i * P:(i + 1) * P],
                        xbs[hf * 2 + i][:, dc * P:(dc + 1) * P], id_sb)
                nc.scalar.activation(
                    out=xt[:, dc, hf * 256:(hf + 1) * 256], in_=tp, func=Copy)

    from contextlib import ExitStack as _ES

    # ---------------- Phase A: K/V projection over all kv rows ----------------
    with _ES() as pa:
        wpool = pa.enter_context(tc.tile_pool(name="pa_w", bufs=1))
        xtpool = pa.enter_context(tc.tile_pool(name="pa_xt", bufs=cfg["pa_xt_bufs"]))
        psA_k = pa.enter_context(
            tc.tile_pool(name="pa_psk", bufs=cfg["pa_psk_bufs"], space="PSUM"))
        psA_v = pa.enter_context(
            tc.tile_pool(name="pa_psv", bufs=cfg["pa_psv_bufs"], space="PSUM"))
        xbp = psA_t = None
        if not dma_xpose:
            xbp = pa.enter_context(
                tc.tile_pool(name="pa_xb", bufs=cfg["pa_xb_bufs"]))
            psA_t = pa.enter_context(
                tc.tile_pool(name="pa_pst", bufs=cfg["pa_pst_bufs"], space="PSUM"))
        wk_sb = wpool.tile([P, 8, 1024], bf16, tag="wk")
        wv_sb = wpool.tile([P, 8, 1024], bf16, tag="wv")
        load_weight(wk, wk_sb)
        load_weight(wv, wv_sb)

        for c in range(NKC_A):
            t0 = c * 512
            xt = xtpool.tile([P, 8, 512], bf16, tag="xt")
            load_xt(xt, xkv, t0, psA_t, xbp, "xb")
            # K^T_[h, t0:t0+512] = Wk^T @ x^T
            for hc in range(8):
                kp = psA_k.tile([P, 512], f32, tag="kp")
                for dc in range(8):
                    nc.tensor.matmul(
                        kp, lhsT=wk_sb[:, dc, hc * P:(hc + 1) * P],
                        rhs=xt[:, dc, :], start=(dc == 0), stop=(dc == 7))
                nc.vector.tensor_copy(out=KT[:, hc, t0:t0 + 512], in_=kp)
            # V_[t0+i*128, :] = x @ Wv
            for i in range(4):
                vp = psA_v.tile([P, 1024], f32, tag="vp")
                for dc in range(8):
                    for nb in range(2):
                        nc.tensor.matmul(
                            vp[:, nb * 512:(nb + 1) * 512],
                            lhsT=xt[:, dc, i * P:(i + 1) * P],
                            rhs=wv_sb[:, dc, nb * 512:(nb + 1) * 512],
                            start=(dc == 0), stop=(dc == 7))
                nc.vector.tensor_copy(out=V[:, t0 // P + i, :], in_=vp)

    # ---------------- Phase B: Q projection + attention ----------------
    with _ES() as pb_es:
        ec = pb_es.enter_context
        xtq_p = ec(tc.tile_pool(name="pb_xt", bufs=cfg["pb_xt_bufs"]))
        qt_p = ec(tc.tile_pool(name="pb_qt", bufs=cfg["pb_qt_bufs"]))
        pb_p = ec(tc.tile_pool(name="pb_p", bufs=cfg["pb_p_bufs"]))
        pt_p = ec(tc.tile_pool(name="pb_pt", bufs=cfg["pb_pt_bufs"]))
        sums_p = ec(tc.tile_pool(name="pb_sums", bufs=2))
        ob_p = ec(tc.tile_pool(name="pb_ob", bufs=cfg["pb_ob_bufs"]))
        ps_pp = ec(tc.tile_pool(name="pb_pp", bufs=cfg["pb_pp_bufs"], space="PSUM"))
        ps_s = ec(tc.tile_pool(name="pb_ps", bufs=cfg["pb_ps_bufs"], space="PSUM"))
        ps_o = ec(tc.tile_pool(name="pb_po", bufs=cfg["pb_po_bufs"], space="PSUM"))
        xbq_p = None
        if not dma_xpose:
            xbq_p = ec(tc.tile_pool(name="pb_xb", bufs=cfg["pb_xb_bufs"]))
        if wq_top:
            wq_sb = wq_sb_top
        else:
            wqp = ec(tc.tile_pool(name="pb_w", bufs=1))
            wq_sb = wqp.tile([P, 8, 1024], bf16, tag="wq")
            load_weight(wq, wq_sb)

        for sc in range(NSC):
            # Q^T for this superchunk: [h%128, h//128, 512 local q]
            xtq = xtq_p.tile([P, 8, 512], bf16, tag="xtq")
            load_xt(xtq, xq, sc * 512, ps_pp, xbq_p, "xbq")
            qt = qt_p.tile([P, 8, 512], bf16, tag="qt")
            for hc in range(8):
                qp = ps_pp.tile([P, 512], f32, tag="pp")
                for dc in range(8):
                    nc.tensor.matmul(
                        qp, lhsT=wq_sb[:, dc, hc * P:(hc + 1) * P],
                        rhs=xtq[:, dc, :], start=(dc == 0), stop=(dc == 7))
                nc.vector.tensor_copy(out=qt[:, hc, :], in_=qp)

            for o in range(4):
                j = sc * 4 + o
                nch = j + 1
                sums = sums_p.tile([P, 16], f32, tag="sums")
                op = ps_o.tile([P, 1024], f32, tag="op")

                def s_mm(c):
                    sp = ps_s.tile([P, 256], f32, tag="sp")
                    for hc in range(8):
                        nc.tensor.matmul(
                            sp, lhsT=qt[:, hc, o * P:(o + 1) * P],
                            rhs=KT[:, hc, c * 256:(c + 1) * 256],
                            start=(hc == 0), stop=(hc == 7))
                    return sp

                def softmax(c, sp):
                    pb = pb_p.tile([P, 256], bf16, tag="pb")
                    if c < nch - 1:
                        nc.scalar.activation(out=pb, in_=sp, func=Exp,
                                             scale=SCALE, accum_out=sums[:, c:c + 1])
                    else:
                        nc.scalar.activation(out=pb, in_=sp, func=Exp, scale=SCALE)
                        nc.vector.tensor_mul(pb, pb, mask_sb)
                        nc.vector.reduce_sum(out=sums[:, c:c + 1], in_=pb, axis=AX)
                    return pb

                def pv(c, pb):
                    ptp = ps_pp.tile([P, 256], bf16, tag="pp")
                    nc.tensor.transpose(ptp[:, 0:P], pb[:, 0:P], id_sb)
                    nc.tensor.transpose(ptp[:, P:256], pb[:, P:256], id_sb)
                    pt = pt_p.tile([P, 256], bf16, tag="pt")
                    nc.vector.tensor_copy(out=pt, in_=ptp)
                    for kl in range(2):
                        kb = c * 2 + kl
                        for nb in range(2):
                            nc.tensor.matmul(
                                op[:, nb * 512:(nb + 1) * 512],
                                lhsT=pt[:, kl * P:(kl + 1) * P],
                                rhs=V[:, kb, nb * 512:(nb + 1) * 512],
                                start=(c == 0 and kl == 0),
                                stop=(c == nch - 1 and kl == 1))

                ahead = cfg["s_ahead"]
                sps, pbs = {}, {}
                for c in range(min(ahead, nch)):
                    sps[c] = s_mm(c)
                    pbs[c] = softmax(c, sps[c])
                for c in range(nch):
                    pv(c, pbs[c])
                    if c + ahead < nch:
                        sps[c + ahead] = s_mm(c + ahead)
                        pbs[c + ahead] = softmax(c + ahead, sps[c + ahead])

                tot = sums_p.tile([P, 1], f32, tag="tot")
                nc.vector.reduce_sum(out=tot, in_=sums[:, 0:nch], axis=AX)
                rec = sums_p.tile([P, 1], f32, tag="rec")
                nc.vector.reciprocal(out=rec, in_=tot)
                ob = ob_p.tile([P, 1024], f32, tag="ob")
                nc.scalar.activation(out=ob, in_=op, func=Copy, scale=rec)
                nc.sync.dma_start(out=outp[j * P:(j + 1) * P, :], in_=ob)


def build_module(T_kv=T, n_qt=None, cfg=None):
    from contextlib import ExitStack
    import concourse.tile as tile
    import concourse.mybir as mybir
    from concourse import bacc

    if n_qt is None:
        n_qt = T_kv // 256
    full_cfg = dict(DEFAULT_CFG)
    if cfg:
        full_cfg.update(cfg)
    cfg = full_cfg
    dt = mybir.dt
    nc = bacc.Bacc("TRN2", target_bir_lowering=False, debug=False,
                   num_devices=NCORES)
    xq = nc.dram_tensor("xq", [n_qt * P, D], dt.bfloat16, kind="ExternalInput").ap()
    xkv = nc.dram_tensor("xkv", [T_kv, D], dt.bfloat16, kind="ExternalInput").ap()
    wq = nc.dram_tensor("wq", [D, H], dt.bfloat16, kind="ExternalInput").ap()
    wk = nc.dram_tensor("wk", [D, H], dt.bfloat16, kind="ExternalInput").ap()
    wv = nc.dram_tensor("wv", [D, H], dt.bfloat16, kind="ExternalInput").ap()
    maskt = nc.dram_tensor("maskt", [P, 256], dt.bfloat16, kind="ExternalInput").ap()
    ident = nc.dram_tensor("ident", [P, P], dt.bfloat16, kind="ExternalInput").ap()
    outp = nc.dram_tensor("outp", [n_qt * P, H], dt.float32, kind="ExternalOutput").ap()

    with tile.TileContext(nc) as tc:
        with ExitStack() as ctx:
            _emit(ctx, tc, xq, xkv, wq, wk, wv, maskt, ident, outp, T_kv, n_qt,
                  cfg)
    nc.compile()
    return nc


def host_inputs(x, Wq, Wk, Wv, T_kv=T, n_qt=None, n_batch=None):
    """Build the per-core input maps for run_bass_kernel_spmd."""
    import ml_dtypes
    bf = ml_dtypes.bfloat16
    if n_qt is None:
        n_qt = T_kv // 256
    if n_batch is None:
        n_batch = x.shape[0]
    eye = np.eye(P, dtype=np.float32).astype(bf)
    tril = np.tril(np.ones((P, P), np.float32))
    m = [np.concatenate([tril, np.zeros((P, P), np.float32)], 1).astype(bf),
         np.concatenate([np.ones((P, P), np.float32), tril], 1).astype(bf)]

    def make_masks(pair):
        return m[pair]
    xb = np.asarray(x, np.float32).astype(bf)
    wqb = np.asarray(Wq, np.float32).astype(bf)
    wkb = np.asarray(Wk, np.float32).astype(bf)
    wvb = np.asarray(Wv, np.float32).astype(bf)
    in_maps = []
    for c in range(NCORES):
        b, pair = (c // 2) % n_batch, c % 2
        qrows = np.concatenate(
            [xb[b, (2 * j + pair) * P:(2 * j + pair + 1) * P, :]
             for j in range(n_qt)], 0)
        in_maps.append({
            "xq": np.ascontiguousarray(qrows),
            "xkv": np.ascontiguousarray(xb[b]),
            "wq": wqb, "wk": wkb, "wv": wvb,
            "maskt": make_masks(pair), "ident": eye,
        })
    return in_maps


def gather_output(results, T_kv=T, n_qt=None, n_batch=B):
    if n_qt is None:
        n_qt = T_kv // 256
    out = np.empty((n_batch, T_kv, H), np.float32)
    for c in range(2 * n_batch):
        b, pair = c // 2, c % 2
        r = results[c]["outp"]
        for j in range(n_qt):
            out[b, (2 * j + pair) * P:(2 * j + pair + 1) * P, :] = \
                r[j * P:(j + 1) * P, :]
    return out


_NC_CACHE = {}


def kernel(x, Wq, Wk, Wv):
    from concourse.bass_utils import run_bass_kernel_spmd

    x = np.asarray(x, dtype=np.float32)
    Wq = np.asarray(Wq, dtype=np.float32)
    Wk = np.asarray(Wk, dtype=np.float32)
    Wv = np.asarray(Wv, dtype=np.float32)

    if "nc" not in _NC_CACHE:
        _NC_CACHE["nc"] = build_module()
    nc = _NC_CACHE["nc"]

    in_maps = host_inputs(x, Wq, Wk, Wv)
    res = run_bass_kernel_spmd(nc, in_maps, core_ids=list(range(NCORES)))
    return gather_output(res.results)



# revision 10
# speedup vs baseline: 164.7170x; 164.7170x over previous
"""Causal self-attention (B=4, T=4096, D=H=1024, fp32) on 8 Trainium2 cores.

Sharding: 2 cores per batch element. Within a batch, the 32 query tiles of
128 rows are interleaved between the 2 cores (core `pair` p takes global
q-tiles p, p+2, p+4, ...), which balances the causal-attention work exactly.
Each core computes the full K/V projection for its batch (replicated between
the 2 cores of a batch), then flash-style attention over its 16 q-tiles.

Numerics: x and the weights are cast to bf16 on the host; all matmuls run
bf16 with fp32 PSUM accumulation. Softmax skips max-subtraction (scores are
~N(0,1) after the 1/32 scale so exp stays in a safe fp32 range); exp runs on
ScalarE in fp32, probabilities are stored bf16, and the final normalization
is fp32. Measured error vs the fp32 reference: ~0.4% scale-relative absmax.
"""

import numpy as np

B, T, D, H = 4, 4096, 1024, 1024
P = 128
NCORES = 8


DEFAULT_CFG = dict(
    phases="AB",
    xpose="pe",           # "pe": PE transposes; "dma": xbar-transpose loads of x^T
    wq_top=False,         # preload Wq before phase A
    fp8_attn=True,        # fp8e4 DoubleRow for scores+PV (except q-tile 0)
    pa_xb_bufs=8, pa_xt_bufs=1,
    pa_pst_bufs=2, pa_psk_bufs=2, pa_psv_bufs=2,
    pb_xb_bufs=8, pb_xt_bufs=1, pb_qt_bufs=1,
    pb_p_bufs=3, pb_pt_bufs=2, pb_ob_bufs=2,
    pb_pp_bufs=3, pb_ps_bufs=3, pb_po_bufs=1,
    s_ahead=2,
)


def _emit(ctx, tc, xq, xkv, wq, wk, wv, maskt, ident, outp, T_kv, n_qt, cfg):
    import concourse.mybir as mybir

    nc = tc.nc
    f32 = mybir.dt.float32
    bf16 = mybir.dt.bfloat16
    f8 = mybir.dt.float8e4
    DR = mybir.MatmulPerfMode.DoubleRow
    Copy = mybir.ActivationFunctionType.Copy
    Exp = mybir.ActivationFunctionType.Exp
    AX = mybir.AxisListType.X
    SCALE = 1.0 / 32.0  # 1/sqrt(H)
    fp8 = cfg["fp8_attn"]

    NKB = T_kv // 128     # kv 128-blocks
    NKC_A = T_kv // 512   # phase-A 512-row projection chunks
    NSC = n_qt // 4       # 512-row query superchunks
    if "A" not in cfg["phases"]:
        NKC_A = 0
    if "B" not in cfg["phases"]:
        NSC = 0

    const = ctx.enter_context(tc.tile_pool(name="const", bufs=1))
    persist = ctx.enter_context(tc.tile_pool(name="persist", bufs=1))

    id_sb = const.tile([P, P], bf16, tag="ident")
    nc.sync.dma_start(out=id_sb, in_=ident)
    mask_sb = const.tile([P, 256], bf16, tag="mask")
    nc.sync.dma_start(out=mask_sb, in_=maskt)

    # K^T laid out [h%128, h//128, t]; V laid out [t%128, t//128, h]
    kv_dt = f8 if fp8 else bf16
    KT = persist.tile([P, 8, T_kv], kv_dt, tag="KT")
    V = persist.tile([P, NKB, 1024], kv_dt, tag="V")
    KT0 = V0 = None
    if fp8:
        # bf16 copies of the first 256 kv rows for the exact q-tile-0 path
        KT0 = persist.tile([P, 8, 256], bf16, tag="KT0")
        V0 = persist.tile([P, 2, 1024], bf16, tag="V0")

    def load_weight(wdram, wsb):
        # DRAM [1024,1024] bf16 -> SBUF [128, 8, 1024] (d = dc*128 + p)
        for dc in range(8):
            nc.sync.dma_start(out=wsb[:, dc, :], in_=wdram[dc * P:(dc + 1) * P, :])

    dma_xpose = cfg["xpose"] == "dma"
    wq_top = cfg["wq_top"]
    if wq_top:
        wq_sb_top = persist.tile([P, 8, 1024], bf16, tag="wq")
        load_weight(wq, wq_sb_top)

    def load_xb(xsrc, r0, xb_pool, xb_tag):
        xbs = []
        for i in range(4):
            xb = xb_pool.tile([P, 1024], bf16, tag=xb_tag)
            nc.sync.dma_start(out=xb, in_=xsrc[r0 + i * P: r0 + (i + 1) * P, :])
            xbs.append(xb)
        return xbs

    def xpose_xb(xt, xbs, ps_pool):
        for hf in range(2):
            for dc in range(8):
                tp = ps_pool.tile([P, 256], bf16, tag="pp")
                for i in range(2):
                    nc.tensor.transpose(
                        tp[:, i * P:(i + 1) * P],
                        xbs[hf * 2 + i][:, dc * P:(dc + 1) * P], id_sb)
                nc.scalar.activation(
                    out=xt[:, dc, hf * 256:(hf + 1) * 256], in_=tp, func=Copy)

    def load_xt(xt, xsrc, r0, ps_pool, xb_pool, xb_tag):
        """Fill xt[:, dc, :] = x[r0:r0+512, dc*128:(dc+1)*128]^T for all dc."""
        if dma_xpose:
            for dc in range(8):
                nc.sync.dma_start_transpose(
                    out=xt[:, dc, :],
                    in_=xsrc[r0:r0 + 512, dc * P:(dc + 1) * P])
            return
        xpose_xb(xt, load_xb(xsrc, r0, xb_pool, xb_tag), ps_pool)

    from contextlib import ExitStack as _ES

    # ---------------- Phase A: K/V projection over all kv rows ----------------
    with _ES() as pa:
        wpool = pa.enter_context(tc.tile_pool(name="pa_w", bufs=1))
        xtpool = pa.enter_context(tc.tile_pool(name="pa_xt", bufs=cfg["pa_xt_bufs"]))
        psA_k = pa.enter_context(
            tc.tile_pool(name="pa_psk", bufs=cfg["pa_psk_bufs"], space="PSUM"))
        psA_v = pa.enter_context(
            tc.tile_pool(name="pa_psv", bufs=cfg["pa_psv_bufs"], space="PSUM"))
        xbp = psA_t = None
        if not dma_xpose:
            xbp = pa.enter_context(
                tc.tile_pool(name="pa_xb", bufs=cfg["pa_xb_bufs"]))
            psA_t = pa.enter_context(
                tc.tile_pool(name="pa_pst", bufs=cfg["pa_pst_bufs"], space="PSUM"))
        # issue chunk-0 x DMAs ahead of the 4.2MB weight load so the PE's
        # first transposes aren't queued behind it
        xbs0 = None
        if not dma_xpose and NKC_A > 0:
            xbs0 = load_xb(xkv, 0, xbp, "xb")
        wk_sb = wpool.tile([P, 8, 1024], bf16, tag="wk")
        wv_sb = wpool.tile([P, 8, 1024], bf16, tag="wv")
        load_weight(wk, wk_sb)
        load_weight(wv, wv_sb)

        for c in range(NKC_A):
            t0 = c * 512
            xt = xtpool.tile([P, 8, 512], bf16, tag="xt")
            if c == 0 and xbs0 is not None:
                xpose_xb(xt, xbs0, psA_t)
            else:
                load_xt(xt, xkv, t0, psA_t, xbp, "xb")
            # K^T_[h, t0:t0+512] = Wk^T @ x^T
            for hc in range(8):
                kp = psA_k.tile([P, 512], f32, tag="kp")
                for dc in range(8):
                    nc.tensor.matmul(
                        kp, lhsT=wk_sb[:, dc, hc * P:(hc + 1) * P],
                        rhs=xt[:, dc, :], start=(dc == 0), stop=(dc == 7))
                nc.vector.tensor_copy(out=KT[:, hc, t0:t0 + 512], in_=kp)
                if fp8 and c == 0:
                    nc.scalar.activation(out=KT0[:, hc, :], in_=kp[:, 0:256],
                                         func=Copy)
            # V_[t0+i*128, :] = x @ Wv
            for i in range(4):
                vp = psA_v.tile([P, 1024], f32, tag="vp")
                for dc in range(8):
                    for nb in range(2):
                        nc.tensor.matmul(
                            vp[:, nb * 512:(nb + 1) * 512],
                            lhsT=xt[:, dc, i * P:(i + 1) * P],
                            rhs=wv_sb[:, dc, nb * 512:(nb + 1) * 512],
                            start=(dc == 0), stop=(dc == 7))
                nc.vector.tensor_copy(out=V[:, t0 // P + i, :], in_=vp)
                if fp8 and c == 0 and i < 2:
                    nc.scalar.activation(out=V0[:, i, :], in_=vp, func=Copy)

    # ---------------- Phase B: Q projection + attention ----------------
    with _ES() as pb_es:
        ec = pb_es.enter_context
        xtq_p = ec(tc.tile_pool(name="pb_xt", bufs=cfg["pb_xt_bufs"]))
        qt_p = ec(tc.tile_pool(name="pb_qt", bufs=cfg["pb_qt_bufs"]))
        pb_p = ec(tc.tile_pool(name="pb_p", bufs=cfg["pb_p_bufs"]))
        pt_p = ec(tc.tile_pool(name="pb_pt", bufs=cfg["pb_pt_bufs"]))
        sums_p = ec(tc.tile_pool(name="pb_sums", bufs=2))
        ob_p = ec(tc.tile_pool(name="pb_ob", bufs=cfg["pb_ob_bufs"]))
        ps_pp = ec(tc.tile_pool(name="pb_pp", bufs=cfg["pb_pp_bufs"], space="PSUM"))
        ps_s = ec(tc.tile_pool(name="pb_ps", bufs=cfg["pb_ps_bufs"], space="PSUM"))
        ps_o = ec(tc.tile_pool(name="pb_po", bufs=cfg["pb_po_bufs"], space="PSUM"))
        xbq_p = None
        if not dma_xpose:
            xbq_p = ec(tc.tile_pool(name="pb_xb", bufs=cfg["pb_xb_bufs"]))
        if wq_top:
            wq_sb = wq_sb_top
        else:
            wqp = ec(tc.tile_pool(name="pb_w", bufs=1))
            wq_sb = wqp.tile([P, 8, 1024], bf16, tag="wq")
            load_weight(wq, wq_sb)

        qt0 = None
        for sc in range(NSC):
            # Q^T for this superchunk: [h%128, h//128, 512 local q]
            xtq = xtq_p.tile([P, 8, 512], bf16, tag="xtq")
            load_xt(xtq, xq, sc * 512, ps_pp, xbq_p, "xbq")
            qt = qt_p.tile([P, 8, 512], f8 if fp8 else bf16, tag="qt")
            if fp8 and sc == 0:
                qt0 = qt_p.tile([P, 8, 128], bf16, tag="qt0")
            for hc in range(8):
                qp = ps_pp.tile([P, 512], f32, tag="pp")
                for dc in range(8):
                    nc.tensor.matmul(
                        qp, lhsT=wq_sb[:, dc, hc * P:(hc + 1) * P],
                        rhs=xtq[:, dc, :], start=(dc == 0), stop=(dc == 7))
                nc.vector.tensor_copy(out=qt[:, hc, :], in_=qp)
                if fp8 and sc == 0:
                    nc.scalar.activation(out=qt0[:, hc, :], in_=qp[:, 0:P],
                                         func=Copy)

            for o in range(4):
                j = sc * 4 + o
                nch = j + 1
                sums = sums_p.tile([P, 16], f32, tag="sums")
                op = ps_o.tile([P, 1024], f32, tag="op")

                def s_mm(c):
                    sp = ps_s.tile([P, 256], f32, tag="sp")
                    if fp8 and j > 0:
                        for hc in range(0, 8, 2):
                            nc.tensor.matmul(
                                sp, lhsT=qt[:, hc:hc + 2, o * P:(o + 1) * P],
                                rhs=KT[:, hc:hc + 2, c * 256:(c + 1) * 256],
                                start=(hc == 0), stop=(hc == 6), perf_mode=DR)
                    elif fp8:
                        # q-tile 0 (rows with few keys): exact bf16 path
                        for hc in range(8):
                            nc.tensor.matmul(
                                sp, lhsT=qt0[:, hc, :], rhs=KT0[:, hc, :],
                                start=(hc == 0), stop=(hc == 7))
                    else:
                        for hc in range(8):
                            nc.tensor.matmul(
                                sp, lhsT=qt[:, hc, o * P:(o + 1) * P],
                                rhs=KT[:, hc, c * 256:(c + 1) * 256],
                                start=(hc == 0), stop=(hc == 7))
                    return sp

                def softmax(c, sp):
                    pb = pb_p.tile([P, 256], bf16, tag="pb")
                    if c < nch - 1:
                        nc.scalar.activation(out=pb, in_=sp, func=Exp,
                                             scale=SCALE, accum_out=sums[:, c:c + 1])
                    else:
                        nc.scalar.activation(out=pb, in_=sp, func=Exp, scale=SCALE)
                        nc.vector.tensor_mul(pb, pb, mask_sb)
                        nc.vector.reduce_sum(out=sums[:, c:c + 1], in_=pb, axis=AX)
                    return pb

                def pv(c, pb):
                    ptp = ps_pp.tile([P, 2, P], bf16, tag="pp")
                    nc.tensor.transpose(ptp[:, 0, :], pb[:, 0:P], id_sb)
                    nc.tensor.transpose(ptp[:, 1, :], pb[:, P:256], id_sb)
                    if fp8 and j > 0:
                        pt = pt_p.tile([P, 2, P], f8, tag="pt")
                        nc.vector.tensor_copy(out=pt, in_=ptp)
                        for nb in range(2):
                            nc.tensor.matmul(
                                op[:, nb * 512:(nb + 1) * 512],
                                lhsT=pt,
                                rhs=V[:, 2 * c:2 * c + 2,
                                      nb * 512:(nb + 1) * 512],
                                start=(c == 0), stop=(c == nch - 1),
                                perf_mode=DR)
                        return
                    pt = pt_p.tile([P, 2, P], bf16, tag="ptb")
                    nc.vector.tensor_copy(out=pt, in_=ptp)
                    VV = V0 if fp8 else V
                    for kl in range(2):
                        kb = c * 2 + kl
                        for nb in range(2):
                            nc.tensor.matmul(
                                op[:, nb * 512:(nb + 1) * 512],
                                lhsT=pt[:, kl, :],
                                rhs=VV[:, kb, nb * 512:(nb + 1) * 512],
                                start=(c == 0 and kl == 0),
                                stop=(c == nch - 1 and kl == 1))

                ahead = cfg["s_ahead"]
                sps, pbs = {}, {}
                for c in range(min(ahead, nch)):
                    sps[c] = s_mm(c)
                    pbs[c] = softmax(c, sps[c])
                for c in range(nch):
                    pv(c, pbs[c])
                    if c + ahead < nch:
                        sps[c + ahead] = s_mm(c + ahead)
                        pbs[c + ahead] = softmax(c + ahead, sps[c + ahead])

                tot = sums_p.tile([P, 1], f32, tag="tot")
                nc.vector.reduce_sum(out=tot, in_=sums[:, 0:nch], axis=AX)
                rec = sums_p.tile([P, 1], f32, tag="rec")
                nc.vector.reciprocal(out=rec, in_=tot)
                ob = ob_p.tile([P, 1024], f32, tag="ob")
                nc.scalar.activation(out=ob, in_=op, func=Copy, scale=rec)
                nc.sync.dma_start(out=outp[j * P:(j + 1) * P, :], in_=ob)


def build_module(T_kv=T, n_qt=None, cfg=None):
    from contextlib import ExitStack
    import concourse.tile as tile
    import concourse.mybir as mybir
    from concourse import bacc

    if n_qt is None:
        n_qt = T_kv // 256
    full_cfg = dict(DEFAULT_CFG)
    if cfg:
        full_cfg.update(cfg)
    cfg = full_cfg
    dt = mybir.dt
    nc = bacc.Bacc("TRN2", target_bir_lowering=False, debug=False,
                   num_devices=NCORES)
    xq = nc.dram_tensor("xq", [n_qt * P, D], dt.bfloat16, kind="ExternalInput").ap()
    xkv = nc.dram_tensor("xkv", [T_kv, D], dt.bfloat16, kind="ExternalInput").ap()
    wq = nc.dram_tensor("wq", [D, H], dt.bfloat16, kind="ExternalInput").ap()
    wk = nc.dram_tensor("wk", [D, H], dt.bfloat16, kind="ExternalInput").ap()
    wv = nc.dram_tensor("wv", [D, H], dt.bfloat16, kind="ExternalInput").ap()
    maskt = nc.dram_tensor("maskt", [P, 256], dt.bfloat16, kind="ExternalInput").ap()
    ident = nc.dram_tensor("ident", [P, P], dt.bfloat16, kind="ExternalInput").ap()
    outp = nc.dram_tensor("outp", [n_qt * P, H], dt.float32, kind="ExternalOutput").ap()

    with tile.TileContext(nc) as tc:
        with ExitStack() as ctx:
            _emit(ctx, tc, xq, xkv, wq, wk, wv, maskt, ident, outp, T_kv, n_qt,
                  cfg)
    nc.compile()
    return nc


def host_inputs(x, Wq, Wk, Wv, T_kv=T, n_qt=None, n_batch=None):
    """Build the per-core input maps for run_bass_kernel_spmd."""
    import ml_dtypes
    bf = ml_dtypes.bfloat16
    if n_qt is None:
        n_qt = T_kv // 256
    if n_batch is None:
        n_batch = x.shape[0]
    eye = np.eye(P, dtype=np.float32).astype(bf)
    tril = np.tril(np.ones((P, P), np.float32))
    m = [np.concatenate([tril, np.zeros((P, P), np.float32)], 1).astype(bf),
         np.concatenate([np.ones((P, P), np.float32), tril], 1).astype(bf)]

    def make_masks(pair):
        return m[pair]
    xb = np.asarray(x, np.float32).astype(bf)
    wqb = np.asarray(Wq, np.float32).astype(bf)
    wkb = np.asarray(Wk, np.float32).astype(bf)
    wvb = np.asarray(Wv, np.float32).astype(bf)
    in_maps = []
    for c in range(NCORES):
        b, pair = (c // 2) % n_batch, c % 2
        qrows = np.concatenate(
            [xb[b, (2 * j + pair) * P:(2 * j + pair + 1) * P, :]
             for j in range(n_qt)], 0)
        in_maps.append({
            "xq": np.ascontiguousarray(qrows),
            "xkv": np.ascontiguousarray(xb[b]),
            "wq": wqb, "wk": wkb, "wv": wvb,
            "maskt": make_masks(pair), "ident": eye,
        })
    return in_maps


def gather_output(results, T_kv=T, n_qt=None, n_batch=B):
    if n_qt is None:
        n_qt = T_kv // 256
    out = np.empty((n_batch, T_kv, H), np.float32)
    for c in range(2 * n_batch):
        b, pair = c // 2, c % 2
        r = results[c]["outp"]
        for j in range(n_qt):
            out[b, (2 * j + pair) * P:(2 * j + pair + 1) * P, :] = \
                r[j * P:(j + 1) * P, :]
    return out


_NC_CACHE = {}


def kernel(x, Wq, Wk, Wv):
    from concourse.bass_utils import run_bass_kernel_spmd

    x = np.asarray(x, dtype=np.float32)
    Wq = np.asarray(Wq, dtype=np.float32)
    Wk = np.asarray(Wk, dtype=np.float32)
    Wv = np.asarray(Wv, dtype=np.float32)

    if "nc" not in _NC_CACHE:
        _NC_CACHE["nc"] = build_module()
    nc = _NC_CACHE["nc"]

    in_maps = host_inputs(x, Wq, Wk, Wv)
    res = run_bass_kernel_spmd(nc, in_maps, core_ids=list(range(NCORES)))
    return gather_output(res.results)



# revision 21
# speedup vs baseline: 196.5869x; 1.1935x over previous
"""Causal self-attention (B=4, T=4096, D=H=1024, fp32) on 8 Trainium2 cores.

Sharding: 2 cores per batch element. Within a batch, the 32 query tiles of
128 rows are interleaved between the 2 cores (core `pair` p takes global
q-tiles p, p+2, p+4, ...), which balances the causal-attention work exactly.
Each core computes the full K/V projection for its batch (replicated between
the 2 cores of a batch), then flash-style attention over its 16 q-tiles.

Numerics: x and the weights are cast to bf16 on the host; all matmuls run
bf16 with fp32 PSUM accumulation. Softmax skips max-subtraction (scores are
~N(0,1) after the 1/32 scale so exp stays in a safe fp32 range); exp runs on
ScalarE in fp32, probabilities are stored bf16, and the final normalization
is fp32. Measured error vs the fp32 reference: ~0.4% scale-relative absmax.
"""

import numpy as np

B, T, D, H = 4, 4096, 1024, 1024
P = 128
NCORES = 8


DEFAULT_CFG = dict(
    phases="AB",
    xpose="pe",           # "pe": PE transposes; "dma": xbar-transpose loads of x^T
    wq_top=False,         # preload Wq before phase A
    fp8_attn=True,        # fp8e4 DoubleRow for scores+PV (except q-tile 0)
    fp8_proj=True,        # fp8e4 DoubleRow K/V proj rows>=1024, Q proj sc>=1
    pa_xb_bufs=8, pa_xt_bufs=1,
    pa_pst_bufs=2, pa_psk_bufs=2, pa_psv_bufs=2,
    pb_xb_bufs=8, pb_xt_bufs=1, pb_qt_bufs=1,
    pb_p_bufs=3, pb_pt_bufs=2, pb_ob_bufs=2,
    pb_pp_bufs=3, pb_ps_bufs=3, pb_po_bufs=1,
    s_ahead=2,
)


def _emit(ctx, tc, xq, xkv, wq, wk, wv, maskt, ident, outp, T_kv, n_qt, cfg):
    import concourse.mybir as mybir

    nc = tc.nc
    f32 = mybir.dt.float32
    bf16 = mybir.dt.bfloat16
    f8 = mybir.dt.float8e4
    DR = mybir.MatmulPerfMode.DoubleRow
    Copy = mybir.ActivationFunctionType.Copy
    Exp = mybir.ActivationFunctionType.Exp
    AX = mybir.AxisListType.X
    SCALE = 1.0 / 32.0  # 1/sqrt(H)
    fp8 = cfg["fp8_attn"]
    fp8p = fp8 and cfg["fp8_proj"]

    NKB = T_kv // 128     # kv 128-blocks
    NKC_A = T_kv // 512   # phase-A 512-row projection chunks
    NSC = n_qt // 4       # 512-row query superchunks
    if "A" not in cfg["phases"]:
        NKC_A = 0
    if "B" not in cfg["phases"]:
        NSC = 0

    const = ctx.enter_context(tc.tile_pool(name="const", bufs=1))
    persist = ctx.enter_context(tc.tile_pool(name="persist", bufs=1))

    id_sb = const.tile([P, P], bf16, tag="ident")
    nc.sync.dma_start(out=id_sb, in_=ident)
    mask_sb = const.tile([P, 256], bf16, tag="mask")
    nc.sync.dma_start(out=mask_sb, in_=maskt)

    # K^T laid out [h%128, h//128, t]; V laid out [t%128, t//128, h]
    kv_dt = f8 if fp8 else bf16
    KT = persist.tile([P, 8, T_kv], kv_dt, tag="KT")
    V = persist.tile([P, NKB, 1024], kv_dt, tag="V")
    KT0 = V0 = None
    if fp8:
        # bf16 copies of the first 1024 kv rows: q-tiles with <1024 keys have
        # concentrated softmax weights, so fp8 quantization noise would land
        # near the 2e-2 gate; they run an exact bf16 path instead
        KT0 = persist.tile([P, 8, 1024], bf16, tag="KT0")
        V0 = persist.tile([P, 8, 1024], bf16, tag="V0")

    def load_weight(wdram, wsb):
        # DRAM [1024,1024] bf16 -> SBUF [128, 8, 1024] (d = dc*128 + p)
        for dc in range(8):
            nc.sync.dma_start(out=wsb[:, dc, :], in_=wdram[dc * P:(dc + 1) * P, :])

    dma_xpose = cfg["xpose"] == "dma"
    wq_top = cfg["wq_top"]
    if wq_top:
        wq_sb_top = persist.tile([P, 8, 1024], bf16, tag="wq")
        load_weight(wq, wq_sb_top)

    def load_xb(xsrc, r0, xb_pool, xb_tag):
        xbs = []
        for i in range(4):
            xb = xb_pool.tile([P, 1024], bf16, tag=xb_tag)
            nc.sync.dma_start(out=xb, in_=xsrc[r0 + i * P: r0 + (i + 1) * P, :])
            xbs.append(xb)
        return xbs

    def xpose_xb(xt, xbs, ps_pool):
        for hf in range(2):
            for dc in range(8):
                tp = ps_pool.tile([P, 256], bf16, tag="pp")
                for i in range(2):
                    nc.tensor.transpose(
                        tp[:, i * P:(i + 1) * P],
                        xbs[hf * 2 + i][:, dc * P:(dc + 1) * P], id_sb)
                nc.scalar.activation(
                    out=xt[:, dc, hf * 256:(hf + 1) * 256], in_=tp, func=Copy)

    def load_xt(xt, xsrc, r0, ps_pool, xb_pool, xb_tag):
        """Fill xt[:, dc, :] = x[r0:r0+512, dc*128:(dc+1)*128]^T for all dc."""
        if dma_xpose:
            for dc in range(8):
                nc.sync.dma_start_transpose(
                    out=xt[:, dc, :],
                    in_=xsrc[r0:r0 + 512, dc * P:(dc + 1) * P])
            return
        xpose_xb(xt, load_xb(xsrc, r0, xb_pool, xb_tag), ps_pool)

    from contextlib import ExitStack as _ES

    # ---------------- Phase A: K/V projection over all kv rows ----------------
    with _ES() as pa:
        wpool = pa.enter_context(tc.tile_pool(name="pa_w", bufs=1))
        xtpool = pa.enter_context(tc.tile_pool(name="pa_xt", bufs=cfg["pa_xt_bufs"]))
        psA_k = pa.enter_context(
            tc.tile_pool(name="pa_psk", bufs=cfg["pa_psk_bufs"], space="PSUM"))
        psA_v = pa.enter_context(
            tc.tile_pool(name="pa_psv", bufs=cfg["pa_psv_bufs"], space="PSUM"))
        xbp = psA_t = None
        if not dma_xpose:
            xbp = pa.enter_context(
                tc.tile_pool(name="pa_xb", bufs=cfg["pa_xb_bufs"]))
            psA_t = pa.enter_context(
                tc.tile_pool(name="pa_pst", bufs=cfg["pa_pst_bufs"], space="PSUM"))
        # issue chunk-0 x DMAs ahead of the 4.2MB weight load so the PE's
        # first transposes aren't queued behind it
        xbs0 = None
        if not dma_xpose and NKC_A > 0:
            xbs0 = load_xb(xkv, 0, xbp, "xb")
        wk_sb = wpool.tile([P, 8, 1024], bf16, tag="wk")
        wv_sb = wpool.tile([P, 8, 1024], bf16, tag="wv")
        load_weight(wk, wk_sb)
        load_weight(wv, wv_sb)
        wk8 = wv8 = None
        if fp8p:
            wk8 = wpool.tile([P, 8, 1024], f8, tag="wk8")
            wv8 = wpool.tile([P, 8, 1024], f8, tag="wv8")
            nc.vector.tensor_copy(out=wk8, in_=wk_sb)
            nc.vector.tensor_copy(out=wv8, in_=wv_sb)

        for c in range(NKC_A):
            t0 = c * 512
            c8 = fp8p and c >= 2
            xt = xtpool.tile([P, 8, 512], f8 if c8 else bf16,
                             tag="xt8" if c8 else "xt")
            if c == 0 and xbs0 is not None:
                xpose_xb(xt, xbs0, psA_t)
            else:
                load_xt(xt, xkv, t0, psA_t, xbp, "xb")
            # K^T_[h, t0:t0+512] = Wk^T @ x^T
            for hc in range(8):
                kp = psA_k.tile([P, 512], f32, tag="kp")
                if c8:
                    for dc in range(0, 8, 2):
                        nc.tensor.matmul(
                            kp, lhsT=wk8[:, dc:dc + 2, hc * P:(hc + 1) * P],
                            rhs=xt[:, dc:dc + 2, :],
                            start=(dc == 0), stop=(dc == 6), perf_mode=DR)
                else:
                    for dc in range(8):
                        nc.tensor.matmul(
                            kp, lhsT=wk_sb[:, dc, hc * P:(hc + 1) * P],
                            rhs=xt[:, dc, :], start=(dc == 0), stop=(dc == 7))
                nc.vector.tensor_copy(out=KT[:, hc, t0:t0 + 512], in_=kp)
                if fp8 and c < 2:
                    nc.scalar.activation(
                        out=KT0[:, hc, t0:t0 + 512], in_=kp, func=Copy)
            # V_[t0+i*128, :] = x @ Wv
            for i in range(4):
                vp = psA_v.tile([P, 1024], f32, tag="vp")
                if c8:
                    for dc in range(0, 8, 2):
                        for nb in range(2):
                            nc.tensor.matmul(
                                vp[:, nb * 512:(nb + 1) * 512],
                                lhsT=xt[:, dc:dc + 2, i * P:(i + 1) * P],
                                rhs=wv8[:, dc:dc + 2, nb * 512:(nb + 1) * 512],
                                start=(dc == 0), stop=(dc == 6), perf_mode=DR)
                else:
                    for dc in range(8):
                        for nb in range(2):
                            nc.tensor.matmul(
                                vp[:, nb * 512:(nb + 1) * 512],
                                lhsT=xt[:, dc, i * P:(i + 1) * P],
                                rhs=wv_sb[:, dc, nb * 512:(nb + 1) * 512],
                                start=(dc == 0), stop=(dc == 7))
                nc.vector.tensor_copy(out=V[:, t0 // P + i, :], in_=vp)
                if fp8 and c < 2:
                    nc.scalar.activation(out=V0[:, t0 // P + i, :], in_=vp,
                                         func=Copy)

    # ---------------- Phase B: Q projection + attention ----------------
    with _ES() as pb_es:
        ec = pb_es.enter_context
        xtq_p = ec(tc.tile_pool(name="pb_xt", bufs=cfg["pb_xt_bufs"]))
        qt_p = ec(tc.tile_pool(name="pb_qt", bufs=cfg["pb_qt_bufs"]))
        pb_p = ec(tc.tile_pool(name="pb_p", bufs=cfg["pb_p_bufs"]))
        pt_p = ec(tc.tile_pool(name="pb_pt", bufs=cfg["pb_pt_bufs"]))
        sums_p = ec(tc.tile_pool(name="pb_sums", bufs=2))
        ob_p = ec(tc.tile_pool(name="pb_ob", bufs=cfg["pb_ob_bufs"]))
        ps_pp = ec(tc.tile_pool(name="pb_pp", bufs=cfg["pb_pp_bufs"], space="PSUM"))
        ps_s = ec(tc.tile_pool(name="pb_ps", bufs=cfg["pb_ps_bufs"], space="PSUM"))
        ps_o = ec(tc.tile_pool(name="pb_po", bufs=cfg["pb_po_bufs"], space="PSUM"))
        xbq_p = None
        if not dma_xpose:
            xbq_p = ec(tc.tile_pool(name="pb_xb", bufs=cfg["pb_xb_bufs"]))
        if wq_top:
            wq_sb = wq_sb_top
        else:
            wqp = ec(tc.tile_pool(name="pb_w", bufs=1))
            wq_sb = wqp.tile([P, 8, 1024], bf16, tag="wq")
            load_weight(wq, wq_sb)
        wq8 = None
        if fp8p:
            wq8 = wqp.tile([P, 8, 1024], f8, tag="wq8")
            nc.vector.tensor_copy(out=wq8, in_=wq_sb)

        qt0 = None
        for sc in range(NSC):
            # Q^T for this superchunk: [h%128, h//128, 512 local q]
            sc8 = fp8p and sc >= 1
            xtq = xtq_p.tile([P, 8, 512], f8 if sc8 else bf16,
                             tag="xtq8" if sc8 else "xtq")
            load_xt(xtq, xq, sc * 512, ps_pp, xbq_p, "xbq")
            qt = qt_p.tile([P, 8, 512], f8 if fp8 else bf16, tag="qt")
            if fp8 and sc == 0:
                qt0 = qt_p.tile([P, 8, 512], bf16, tag="qt0")
            for hc in range(8):
                qp = ps_pp.tile([P, 512], f32, tag="pp")
                if sc8:
                    for dc in range(0, 8, 2):
                        nc.tensor.matmul(
                            qp, lhsT=wq8[:, dc:dc + 2, hc * P:(hc + 1) * P],
                            rhs=xtq[:, dc:dc + 2, :],
                            start=(dc == 0), stop=(dc == 6), perf_mode=DR)
                else:
                    for dc in range(8):
                        nc.tensor.matmul(
                            qp, lhsT=wq_sb[:, dc, hc * P:(hc + 1) * P],
                            rhs=xtq[:, dc, :], start=(dc == 0), stop=(dc == 7))
                nc.vector.tensor_copy(out=qt[:, hc, :], in_=qp)
                if fp8 and sc == 0:
                    nc.scalar.activation(out=qt0[:, hc, :], in_=qp, func=Copy)

            for o in range(4):
                j = sc * 4 + o
                nch = j + 1
                sums = sums_p.tile([P, 16], f32, tag="sums")
                op = ps_o.tile([P, 1024], f32, tag="op")

                def s_mm(c):
                    sp = ps_s.tile([P, 256], f32, tag="sp")
                    if fp8 and j > 3:
                        for hc in range(0, 8, 2):
                            nc.tensor.matmul(
                                sp, lhsT=qt[:, hc:hc + 2, o * P:(o + 1) * P],
                                rhs=KT[:, hc:hc + 2, c * 256:(c + 1) * 256],
                                start=(hc == 0), stop=(hc == 6), perf_mode=DR)
                    elif fp8:
                        # q-tiles with <1024 keys: exact bf16 path
                        for hc in range(8):
                            nc.tensor.matmul(
                                sp, lhsT=qt0[:, hc, o * P:(o + 1) * P],
                                rhs=KT0[:, hc, c * 256:(c + 1) * 256],
                                start=(hc == 0), stop=(hc == 7))
                    else:
                        for hc in range(8):
                            nc.tensor.matmul(
                                sp, lhsT=qt[:, hc, o * P:(o + 1) * P],
                                rhs=KT[:, hc, c * 256:(c + 1) * 256],
                                start=(hc == 0), stop=(hc == 7))
                    return sp

                def softmax(c, sp):
                    pb = pb_p.tile([P, 256], bf16, tag="pb")
                    if c < nch - 1:
                        nc.scalar.activation(out=pb, in_=sp, func=Exp,
                                             scale=SCALE, accum_out=sums[:, c:c + 1])
                    else:
                        nc.scalar.activation(out=pb, in_=sp, func=Exp, scale=SCALE)
                        nc.vector.tensor_mul(pb, pb, mask_sb)
                        nc.vector.reduce_sum(out=sums[:, c:c + 1], in_=pb, axis=AX)
                    return pb

                def pv(c, pb):
                    ptp = ps_pp.tile([P, 2, P], bf16, tag="pp")
                    nc.tensor.transpose(ptp[:, 0, :], pb[:, 0:P], id_sb)
                    nc.tensor.transpose(ptp[:, 1, :], pb[:, P:256], id_sb)
                    if fp8 and j > 3:
                        pt = pt_p.tile([P, 2, P], f8, tag="pt")
                        nc.vector.tensor_copy(out=pt, in_=ptp)
                        for nb in range(2):
                            nc.tensor.matmul(
                                op[:, nb * 512:(nb + 1) * 512],
                                lhsT=pt,
                                rhs=V[:, 2 * c:2 * c + 2,
                                      nb * 512:(nb + 1) * 512],
                                start=(c == 0), stop=(c == nch - 1),
                                perf_mode=DR)
                        return
                    pt = pt_p.tile([P, 2, P], bf16, tag="ptb")
                    nc.vector.tensor_copy(out=pt, in_=ptp)
                    VV = V0 if fp8 else V
                    for kl in range(2):
                        kb = c * 2 + kl
                        for nb in range(2):
                            nc.tensor.matmul(
                                op[:, nb * 512:(nb + 1) * 512],
                                lhsT=pt[:, kl, :],
                                rhs=VV[:, kb, nb * 512:(nb + 1) * 512],
                                start=(c == 0 and kl == 0),
                                stop=(c == nch - 1 and kl == 1))

                ahead = cfg["s_ahead"]
                sps, pbs = {}, {}
                for c in range(min(ahead, nch)):
                    sps[c] = s_mm(c)
                    pbs[c] = softmax(c, sps[c])
                for c in range(nch):
                    pv(c, pbs[c])
                    if c + ahead < nch:
                        sps[c + ahead] = s_mm(c + ahead)
                        pbs[c + ahead] = softmax(c + ahead, sps[c + ahead])

                tot = sums_p.tile([P, 1], f32, tag="tot")
                nc.vector.reduce_sum(out=tot, in_=sums[:, 0:nch], axis=AX)
                rec = sums_p.tile([P, 1], f32, tag="rec")
                nc.vector.reciprocal(out=rec, in_=tot)
                ob = ob_p.tile([P, 1024], f32, tag="ob")
                nc.scalar.activation(out=ob, in_=op, func=Copy, scale=rec)
                nc.sync.dma_start(out=outp[j * P:(j + 1) * P, :], in_=ob)


def build_module(T_kv=T, n_qt=None, cfg=None):
    from contextlib import ExitStack
    import concourse.tile as tile
    import concourse.mybir as mybir
    from concourse import bacc

    if n_qt is None:
        n_qt = T_kv // 256
    full_cfg = dict(DEFAULT_CFG)
    if cfg:
        full_cfg.update(cfg)
    cfg = full_cfg
    dt = mybir.dt
    nc = bacc.Bacc("TRN2", target_bir_lowering=False, debug=False,
                   num_devices=NCORES)
    xq = nc.dram_tensor("xq", [n_qt * P, D], dt.bfloat16, kind="ExternalInput").ap()
    xkv = nc.dram_tensor("xkv", [T_kv, D], dt.bfloat16, kind="ExternalInput").ap()
    wq = nc.dram_tensor("wq", [D, H], dt.bfloat16, kind="ExternalInput").ap()
    wk = nc.dram_tensor("wk", [D, H], dt.bfloat16, kind="ExternalInput").ap()
    wv = nc.dram_tensor("wv", [D, H], dt.bfloat16, kind="ExternalInput").ap()
    maskt = nc.dram_tensor("maskt", [P, 256], dt.bfloat16, kind="ExternalInput").ap()
    ident = nc.dram_tensor("ident", [P, P], dt.bfloat16, kind="ExternalInput").ap()
    outp = nc.dram_tensor("outp", [n_qt * P, H], dt.float32, kind="ExternalOutput").ap()

    with tile.TileContext(nc) as tc:
        with ExitStack() as ctx:
            _emit(ctx, tc, xq, xkv, wq, wk, wv, maskt, ident, outp, T_kv, n_qt,
                  cfg)
    nc.compile()
    return nc


def host_inputs(x, Wq, Wk, Wv, T_kv=T, n_qt=None, n_batch=None):
    """Build the per-core input maps for run_bass_kernel_spmd."""
    import ml_dtypes
    bf = ml_dtypes.bfloat16
    if n_qt is None:
        n_qt = T_kv // 256
    if n_batch is None:
        n_batch = x.shape[0]
    eye = np.eye(P, dtype=np.float32).astype(bf)
    tril = np.tril(np.ones((P, P), np.float32))
    m = [np.concatenate([tril, np.zeros((P, P), np.float32)], 1).astype(bf),
         np.concatenate([np.ones((P, P), np.float32), tril], 1).astype(bf)]

    def make_masks(pair):
        return m[pair]
    xb = np.asarray(x, np.float32).astype(bf)
    wqb = np.asarray(Wq, np.float32).astype(bf)
    wkb = np.asarray(Wk, np.float32).astype(bf)
    wvb = np.asarray(Wv, np.float32).astype(bf)
    in_maps = []
    for c in range(NCORES):
        b, pair = (c // 2) % n_batch, c % 2
        qrows = np.concatenate(
            [xb[b, (2 * j + pair) * P:(2 * j + pair + 1) * P, :]
             for j in range(n_qt)], 0)
        in_maps.append({
            "xq": np.ascontiguousarray(qrows),
            "xkv": np.ascontiguousarray(xb[b]),
            "wq": wqb, "wk": wkb, "wv": wvb,
            "maskt": make_masks(pair), "ident": eye,
        })
    return in_maps


def gather_output(results, T_kv=T, n_qt=None, n_batch=B):
    if n_qt is None:
        n_qt = T_kv // 256
    out = np.empty((n_batch, T_kv, H), np.float32)
    for c in range(2 * n_batch):
        b, pair = c // 2, c % 2
        r = results[c]["outp"]
        for j in range(n_qt):
            out[b, (2 * j + pair) * P:(2 * j + pair + 1) * P, :] = \
                r[j * P:(j + 1) * P, :]
    return out


_NC_CACHE = {}


def kernel(x, Wq, Wk, Wv):
    from concourse.bass_utils import run_bass_kernel_spmd

    x = np.asarray(x, dtype=np.float32)
    Wq = np.asarray(Wq, dtype=np.float32)
    Wk = np.asarray(Wk, dtype=np.float32)
    Wv = np.asarray(Wv, dtype=np.float32)

    if "nc" not in _NC_CACHE:
        _NC_CACHE["nc"] = build_module()
    nc = _NC_CACHE["nc"]

    in_maps = host_inputs(x, Wq, Wk, Wv)
    res = run_bass_kernel_spmd(nc, in_maps, core_ids=list(range(NCORES)))
    return gather_output(res.results)

